# revision 14
# baseline (speedup 1.0000x reference)
"""Trainium2 Bass kernel for nn_DuhamelLayer (8-channel long-FIR conv1d).

Math: out[b,o,t] = sum_k irf[o,k] * x[b, t+k-pad]  (cross-correlation,
'SAME' padding, pad = MAXK//2).  The conv is recast as a chain of
PSUM-accumulating 128x128 Toeplitz-block matmuls on the TensorEngine:

  t = 128*a + p,  k = 128*c + (u - p) - PADOFF
  out[p, a] = sum_c sum_u M_c[u, p] * X[u, a + c]
  M_c[u, p] = w[128*c + u - p - PADOFF]      (dense Toeplitz block)
  X[u, m]   = xpad[128*m + u]                (partition-fast input layout)
  xpad      = [PAD+PADOFF zeros | x | tail zeros]

PADOFF=78 realigns every channel's tap band to the 128-grid so only 62
blocks (vs 66 at natural alignment) have any nonzero tap.

Precision split (rel-l2 budget 2e-2, measured ~7e-3): high-energy
channels 0-3 run bf16 (1 cyc/row, FWL weight loads); low-energy
channels 4-7 (3% of output energy) run fp8-e4m3 with
perf_mode=DoubleRow, which packs two Toeplitz blocks into one matmul
at 0.5 cyc/row.  Output is stored bf16 and upcast on host.

Sharding: data-parallel over batch, 2 batches per core x 8 cores.
"""

import numpy as np

# ---- static config (mirrors the nn.Module) ----
OMEGAS = [5.0, 7.0, 9.0, 12.0, 16.0, 22.0, 30.0, 40.0]
XI = 0.05
DT = 0.01
UJ_U1 = 0.01

_decay = (1.0 / (2.0 * np.pi * XI)) * np.log(1.0 / UJ_U1)
VALID_W = [int(2.0 * np.pi / w / np.sqrt(1.0 - XI**2) * _decay / DT) for w in OMEGAS]
KER = [2 * a - 1 for a in VALID_W]
MAXK = max(KER)          # 3687
OUT_CH = len(OMEGAS)     # 8
PAD = MAXK // 2          # 1843

B = 16                   # batch
T = 65536                # sequence length
NCORES = 8
BPC = B // NCORES        # 2 batches per core
A = T // 128             # 512 output columns per (b, o) tile

# block-grid realignment: taps k map to diagonal 128c + u - p - PADOFF.
# PADOFF=78 (i.e. shift s=50 plus one leading zero block) minimizes the
# total number of nonzero Toeplitz blocks: 62 vs 66.
PADOFF = 78
PADX = PAD + PADOFF      # leading zeros in xpad

FP8_CH = (4, 5, 6, 7)    # channels computed in fp8 DoubleRow (3% energy)
SX = 32.0                # fp8 input scale (|x*32| < 240 for randn input)
NWARM = 8                # PE warm-up matmuls bridging the input-DMA wait

MM_DTYPE = "bf16+fp8dr"  # informational (test.py prints it)
MODE = "tile"
TRACE = False            # test.py flips this for profiling
TRACE_KWARGS = {}
LAST_RESULTS = None

_NC_CACHE = {}


def _build_wbank(log_omegas):
    """float32 numpy mirror of the reference's _build_irfs -> [OUT_CH, MAXK]."""
    lo = np.asarray(log_omegas, dtype=np.float32)
    omegas = np.clip(np.exp(lo), 0.01, 1000.0).astype(np.float32)
    sq = np.float32(np.sqrt(np.float32(1.0 - XI**2)))
    rows = []
    for i in range(OUT_CH):
        W, K = VALID_W[i], KER[i]
        tt = (np.arange(W, dtype=np.float32) * np.float32(DT)).astype(np.float32)
        omegaD = np.float32(omegas[i] * sq)
        irf = (
            (np.float32(1.0) / omegaD)
            * np.exp((-np.float32(XI) * omegas[i]) * tt)
            * np.sin(omegaD * tt)
        ).astype(np.float32)
        w = np.concatenate([irf[::-1], np.zeros((K // 2,), np.float32)])
        addpad = MAXK - K
        w = np.pad(w, (addpad // 2, addpad // 2))
        rows.append(w)
    return np.stack(rows)


def _plan_blocks(wbank):
    """Per channel, the list of Toeplitz block indices c with any nonzero tap.

    Tap k contributes to block c = (k + p + PADOFF)//128 for p in [0,128),
    so channel o needs c in [(kmin+PADOFF)//128, (kmax+127+PADOFF)//128].
    """
    blocks = []
    for o in range(OUT_CH):
        nz = np.nonzero(wbank[o])[0]
        kmin, kmax = int(nz.min()), int(nz.max())
        c_lo = (kmin + PADOFF) // 128
        c_hi = (kmax + 127 + PADOFF) // 128
        blocks.append(list(range(c_lo, c_hi + 1)))
    return blocks


def _toeplitz_block(wrow, c):
    u = np.arange(128)[:, None]
    p = np.arange(128)[None, :]
    idx = 128 * c + u - p - PADOFF
    valid = (idx >= 0) & (idx < MAXK)
    return np.where(valid, wrow[np.clip(idx, 0, MAXK - 1)], np.float32(0.0))


def _build_weights(wbank, blocks):
    """bf16 mats for channels 0-3, fp8 pair-mats + descales for 4-7."""
    import ml_dtypes

    bf16 = np.dtype(ml_dtypes.bfloat16)
    e4m3 = np.dtype(ml_dtypes.float8_e4m3fn)
    wmats, w8mats, descale = {}, {}, {}
    for o in range(OUT_CH):
        cols = [_toeplitz_block(wbank[o], c) for c in blocks[o]]
        if o in FP8_CH:
            sw = 2.0 ** np.floor(np.log2(224.0 / np.abs(wbank[o]).max()))
            if len(cols) % 2:
                cols.append(np.zeros((128, 128), np.float32))
            m = np.concatenate(cols, axis=1) * np.float32(sw)
            w8mats[o] = np.clip(m, -240, 240).astype(e4m3)
            descale[o] = 1.0 / (SX * sw)
        else:
            wmats[o] = np.ascontiguousarray(
                np.concatenate(cols, axis=1)
            ).astype(bf16)
    return wmats, w8mats, descale


def _build_nc(blocks, xcols, descale):
    import concourse.bacc as bacc
    import concourse.mybir as mybir
    from concourse.ap import AP
    from concourse.tile import TileContext

    bf16 = mybir.dt.bfloat16
    f8 = mybir.dt.float8e4
    f32 = mybir.dt.float32
    DR = mybir.MatmulPerfMode.DoubleRow

    bf_ch = [o for o in range(OUT_CH) if o not in FP8_CH]
    npairs = {o: (len(blocks[o]) + 1) // 2 for o in FP8_CH}

    # fp8 channels first (smallest weight DMAs -> earliest stream start),
    # then bf16 channels smallest-first.
    fp8_order = sorted(FP8_CH, key=lambda o: npairs[o])
    order = fp8_order + sorted(bf_ch, key=lambda o: len(blocks[o]))
    # fp8 weights ship as two grouped DMAs (first two channels, then the
    # rest): one DMA per channel loses ~850ns apiece to issue+receipt
    # latency at the stream head, one DMA for all four lands too late.
    w8_groups = [fp8_order[:2], fp8_order[2:]]
    w8_base = {}
    for grp in w8_groups:
        base = 0
        for o in grp:
            w8_base[o] = base
            base += npairs[o]

    nc = bacc.Bacc("TRN2", target_bir_lowering=False, debug=False)
    x8_d = nc.dram_tensor("x8", [128, BPC * xcols], f8, kind="ExternalInput")
    xb_d = nc.dram_tensor("xb", [128, BPC * xcols], bf16, kind="ExternalInput")
    w8_d = [
        nc.dram_tensor(
            f"w8g{g}", [128, sum(npairs[o] for o in grp) * 256], f8,
            kind="ExternalInput",
        )
        for g, grp in enumerate(w8_groups)
    ]
    wb_d = {
        o: nc.dram_tensor(f"wt{o}", [128, len(blocks[o]) * 128], bf16, kind="ExternalInput")
        for o in bf_ch
    }
    y_d = nc.dram_tensor("y", [BPC, OUT_CH, 128, A], bf16, kind="ExternalOutput")

    def dr_weight_ap(wt, o, i):
        a = wt[:]
        off = (w8_base[o] + i) * 256
        return AP(a.tensor, a.offset + off, [list(a.ap[0]), [128, 2], [1, 128]])

    def dr_moving_ap(xt, b, c):
        a = xt[:]
        off = b * xcols + c
        return AP(a.tensor, a.offset + off, [list(a.ap[0]), [1, 2], [1, A]])

    with TileContext(nc) as tc:
        with (
            tc.tile_pool(name="w", bufs=1) as wpool,
            tc.tile_pool(name="x", bufs=1) as xpool,
            tc.tile_pool(name="warm", bufs=1) as warmpool,
            tc.tile_pool(name="ps", bufs=4, space="PSUM") as pspool,
            tc.tile_pool(name="o", bufs=4) as opool,
        ):
            # PE warm-up: dependency-free matmuls bridge the input-DMA wait
            # so HAM un-throttles (1.2 -> 2.4 GHz) before the real stream.
            # The warm-up PSUM tile comes from the ps0 rotation so all 8
            # PSUM banks serve the real chains (4 in flight per batch).
            warm = warmpool.tile([128, 384], bf16, name="warm")
            nc.vector.memset(warm[:], 0.0)
            warm_ps = pspool.tile([128, A], f32, tag="ps0", name="warmps")
            for _ in range(NWARM):
                nc.tensor.matmul(
                    warm_ps[:, :256], warm[:, :128], warm[:, 128:384],
                    start=True, stop=True,
                )
            # DMAs split across both HWDGE rings (sync: x + b0 outputs,
            # scalar: weights + b1 outputs) -- each issue costs ~600ns
            # serially per ring, so the x and first-weight loads must not
            # queue behind each other.
            x8t = xpool.tile([128, BPC * xcols], f8, name="x8")
            xbt = xpool.tile([128, BPC * xcols], bf16, name="xb")
            nc.sync.dma_start(x8t[:], x8_d[:])
            nc.sync.dma_start(xbt[:], xb_d[:])
            w8tiles = []
            for g, grp in enumerate(w8_groups):
                wt = wpool.tile(
                    [128, sum(npairs[o] for o in grp) * 256], f8,
                    tag=f"w8g{g}", name=f"w8g{g}",
                )
                nc.scalar.dma_start(wt[:], w8_d[g][:])
                w8tiles.append(wt)
            w8tile = {o: w8tiles[g] for g, grp in enumerate(w8_groups) for o in grp}
            wtiles = {}
            for o in order:
                if o in FP8_CH:
                    continue
                wt = wpool.tile(
                    [128, len(blocks[o]) * 128], bf16, tag=f"w{o}", name=f"w{o}"
                )
                nc.scalar.dma_start(wt[:], wb_d[o][:])
                wtiles[o] = wt

            for k, o in enumerate(order):
                cs = blocks[o]
                last = k == len(order) - 1
                pss = [
                    pspool.tile([128, A], f32, tag=f"ps{b}", name=f"ps{o}_{b}")
                    for b in range(BPC)
                ]
                # block-major, batches inner: each weight block (or DR
                # pair) loads into the PE array once and streams both
                # batches' columns, keeping LDWEIGHTS fully hidden.
                if o in FP8_CH:
                    for i in range(npairs[o]):
                        for b in range(BPC):
                            nc.tensor.matmul(
                                pss[b][:],
                                dr_weight_ap(w8tile[o], o, i),
                                dr_moving_ap(x8t, b, cs[2 * i]),
                                start=(i == 0),
                                stop=(i == npairs[o] - 1),
                                perf_mode=DR,
                            )
                else:
                    for i, c in enumerate(cs):
                        for b in range(BPC):
                            nc.tensor.matmul(
                                pss[b][:],
                                wtiles[o][:, i * 128 : (i + 1) * 128],
                                xbt[:, b * xcols + c : b * xcols + c + A],
                                start=(i == 0),
                                stop=(i == len(cs) - 1),
                            )
                # b0 casts on vector + b0 output DMAs on the sync ring;
                # b1 casts on scalar (ACT) + b1 DMAs on the scalar ring:
                # the two output paths run fully in parallel, so the tail
                # after the last matmul is one cast + one DMA deep.
                scale = float(descale[o]) if o in FP8_CH else 1.0
                for b in range(BPC):
                    ot = opool.tile([128, A], bf16, tag=f"ot{b}", name=f"ot{o}_{b}")
                    if b == 0:
                        if o in FP8_CH:
                            nc.vector.tensor_scalar_mul(ot[:], pss[b][:], scale)
                        else:
                            nc.vector.tensor_copy(ot[:], pss[b][:])
                        nc.sync.dma_start(y_d[b, o], ot[:])
                    else:
                        nc.scalar.activation(
                            ot[:],
                            pss[b][:],
                            mybir.ActivationFunctionType.Copy,
                            scale=scale,
                        )
                        nc.scalar.dma_start(y_d[b, o], ot[:])
    nc.compile()
    return nc


def kernel(inputs, log_omegas):
    global LAST_RESULTS
    import ml_dtypes
    from concourse.bass_utils import run_bass_kernel_spmd

    bf16 = np.dtype(ml_dtypes.bfloat16)
    e4m3 = np.dtype(ml_dtypes.float8_e4m3fn)

    x = np.asarray(inputs, dtype=np.float32).reshape(B, T)
    wbank = _build_wbank(log_omegas)
    blocks = _plan_blocks(wbank)
    cmax = max(c for cs in blocks for c in cs)
    xcols = A + cmax + 1
    assert PADX + T <= xcols * 128, "input padding does not fit block reach"
    wmats, w8mats, descale = _build_weights(wbank, blocks)

    # X[b][u, m] = xpad[b][128*m + u], xpad = [PADX zeros | x | tail zeros]
    xpad = np.zeros((B, xcols * 128), np.float32)
    xpad[:, PADX : PADX + T] = x
    xt_all = np.ascontiguousarray(
        xpad.reshape(B, xcols, 128).transpose(0, 2, 1)
    )  # [B, 128, xcols] f32
    # per core: both batches side by side in the free dim -> [128, BPC*xcols]
    xt_core = np.ascontiguousarray(
        xt_all.reshape(NCORES, BPC, 128, xcols).transpose(0, 2, 1, 3).reshape(
            NCORES, 128, BPC * xcols
        )
    )
    xb_core = xt_core.astype(bf16)
    x8_core = np.clip(xt_core * np.float32(SX), -240, 240).astype(e4m3)

    key = (tuple(tuple(cs) for cs in blocks), xcols, tuple(sorted(descale.items())))
    if key not in _NC_CACHE:
        _NC_CACHE[key] = _build_nc(blocks, xcols, descale)
    nc = _NC_CACHE[key]

    fp8_order = sorted(FP8_CH, key=lambda o: (len(blocks[o]) + 1) // 2)
    w8g = [
        np.ascontiguousarray(np.concatenate([w8mats[o] for o in grp], axis=1))
        for grp in (fp8_order[:2], fp8_order[2:])
    ]
    in_maps = []
    for i in range(NCORES):
        m = {"x8": x8_core[i], "xb": xb_core[i], "w8g0": w8g[0], "w8g1": w8g[1]}
        for o in range(OUT_CH):
            if o not in FP8_CH:
                m[f"wt{o}"] = wmats[o]
        in_maps.append(m)

    res = run_bass_kernel_spmd(
        nc, in_maps, list(range(NCORES)), trace=TRACE, **TRACE_KWARGS
    )
    LAST_RESULTS = res

    # y_dev[b_loc, o, p, a] = y[b, o, 128*a + p]
    y = np.empty((B, OUT_CH, T), np.float32)
    for i in range(NCORES):
        arr = np.asarray(res.results[i]["y"]).astype(np.float32)
        for b in range(BPC):
            y[i * BPC + b] = arr[b].transpose(0, 2, 1).reshape(OUT_CH, T)
    return y.reshape(B, OUT_CH, T)


# revision 16
# speedup vs baseline: 1.0310x; 1.0310x over previous
"""Trainium2 Bass kernel for nn_DuhamelLayer (8-channel long-FIR conv1d).

Math: out[b,o,t] = sum_k irf[o,k] * x[b, t+k-pad]  (cross-correlation,
'SAME' padding, pad = MAXK//2).  The conv is recast as a chain of
PSUM-accumulating 128x128 Toeplitz-block matmuls on the TensorEngine:

  t = 128*a + p,  k = 128*c + (u - p) - PADOFF
  out[p, a] = sum_c sum_u M_c[u, p] * X[u, a + c]
  M_c[u, p] = w[128*c + u - p - PADOFF]      (dense Toeplitz block)
  X[u, m]   = xpad[128*m + u]                (partition-fast input layout)
  xpad      = [PAD+PADOFF zeros | x | tail zeros]

PADOFF=78 realigns every channel's tap band to the 128-grid so only 62
blocks (vs 66 at natural alignment) have any nonzero tap.

Precision split (rel-l2 budget 2e-2, measured ~7e-3): high-energy
channels 0-3 run bf16 (1 cyc/row, FWL weight loads); low-energy
channels 4-7 (3% of output energy) run fp8-e4m3 with
perf_mode=DoubleRow, which packs two Toeplitz blocks into one matmul
at 0.5 cyc/row.  Output is stored bf16 and upcast on host.

Sharding: data-parallel over batch, 2 batches per core x 8 cores.
"""

import numpy as np

# ---- static config (mirrors the nn.Module) ----
OMEGAS = [5.0, 7.0, 9.0, 12.0, 16.0, 22.0, 30.0, 40.0]
XI = 0.05
DT = 0.01
UJ_U1 = 0.01

_decay = (1.0 / (2.0 * np.pi * XI)) * np.log(1.0 / UJ_U1)
VALID_W = [int(2.0 * np.pi / w / np.sqrt(1.0 - XI**2) * _decay / DT) for w in OMEGAS]
KER = [2 * a - 1 for a in VALID_W]
MAXK = max(KER)          # 3687
OUT_CH = len(OMEGAS)     # 8
PAD = MAXK // 2          # 1843

B = 16                   # batch
T = 65536                # sequence length
NCORES = 8
BPC = B // NCORES        # 2 batches per core
A = T // 128             # 512 output columns per (b, o) tile

# block-grid realignment: taps k map to diagonal 128c + u - p - PADOFF.
# PADOFF=78 (i.e. shift s=50 plus one leading zero block) minimizes the
# total number of nonzero Toeplitz blocks: 62 vs 66.
PADOFF = 78
PADX = PAD + PADOFF      # leading zeros in xpad

FP8_CH = (3, 4, 5, 6, 7)  # channels computed in fp8 DoubleRow (7% energy)
SX = 32.0                # fp8 input scale (|x*32| < 240 for randn input)
NWARM = 11               # PE warm-up matmuls bridging the input-DMA wait

MM_DTYPE = "bf16+fp8dr"  # informational (test.py prints it)
MODE = "tile"
TRACE = False            # test.py flips this for profiling
TRACE_KWARGS = {}
LAST_RESULTS = None

_NC_CACHE = {}


def _build_wbank(log_omegas):
    """float32 numpy mirror of the reference's _build_irfs -> [OUT_CH, MAXK]."""
    lo = np.asarray(log_omegas, dtype=np.float32)
    omegas = np.clip(np.exp(lo), 0.01, 1000.0).astype(np.float32)
    sq = np.float32(np.sqrt(np.float32(1.0 - XI**2)))
    rows = []
    for i in range(OUT_CH):
        W, K = VALID_W[i], KER[i]
        tt = (np.arange(W, dtype=np.float32) * np.float32(DT)).astype(np.float32)
        omegaD = np.float32(omegas[i] * sq)
        irf = (
            (np.float32(1.0) / omegaD)
            * np.exp((-np.float32(XI) * omegas[i]) * tt)
            * np.sin(omegaD * tt)
        ).astype(np.float32)
        w = np.concatenate([irf[::-1], np.zeros((K // 2,), np.float32)])
        addpad = MAXK - K
        w = np.pad(w, (addpad // 2, addpad // 2))
        rows.append(w)
    return np.stack(rows)


def _plan_blocks(wbank):
    """Per channel, the list of Toeplitz block indices c with any nonzero tap.

    Tap k contributes to block c = (k + p + PADOFF)//128 for p in [0,128),
    so channel o needs c in [(kmin+PADOFF)//128, (kmax+127+PADOFF)//128].
    """
    blocks = []
    for o in range(OUT_CH):
        nz = np.nonzero(wbank[o])[0]
        kmin, kmax = int(nz.min()), int(nz.max())
        c_lo = (kmin + PADOFF) // 128
        c_hi = (kmax + 127 + PADOFF) // 128
        blocks.append(list(range(c_lo, c_hi + 1)))
    return blocks


def _toeplitz_block(wrow, c):
    u = np.arange(128)[:, None]
    p = np.arange(128)[None, :]
    idx = 128 * c + u - p - PADOFF
    valid = (idx >= 0) & (idx < MAXK)
    return np.where(valid, wrow[np.clip(idx, 0, MAXK - 1)], np.float32(0.0))


def _build_weights(wbank, blocks):
    """bf16 mats for channels 0-3, fp8 pair-mats + descales for 4-7."""
    import ml_dtypes

    bf16 = np.dtype(ml_dtypes.bfloat16)
    e4m3 = np.dtype(ml_dtypes.float8_e4m3fn)
    wmats, w8mats, descale = {}, {}, {}
    for o in range(OUT_CH):
        cols = [_toeplitz_block(wbank[o], c) for c in blocks[o]]
        if o in FP8_CH:
            sw = 2.0 ** np.floor(np.log2(224.0 / np.abs(wbank[o]).max()))
            if len(cols) % 2:
                cols.append(np.zeros((128, 128), np.float32))
            m = np.concatenate(cols, axis=1) * np.float32(sw)
            w8mats[o] = np.clip(m, -240, 240).astype(e4m3)
            descale[o] = 1.0 / (SX * sw)
        else:
            wmats[o] = np.ascontiguousarray(
                np.concatenate(cols, axis=1)
            ).astype(bf16)
    return wmats, w8mats, descale


def _build_nc(blocks, xcols, descale):
    import concourse.bacc as bacc
    import concourse.mybir as mybir
    from concourse.ap import AP
    from concourse.tile import TileContext

    bf16 = mybir.dt.bfloat16
    f8 = mybir.dt.float8e4
    f32 = mybir.dt.float32
    DR = mybir.MatmulPerfMode.DoubleRow

    bf_ch = [o for o in range(OUT_CH) if o not in FP8_CH]
    npairs = {o: (len(blocks[o]) + 1) // 2 for o in FP8_CH}

    # fp8 channels first (smallest weight DMAs -> earliest stream start),
    # then bf16 channels smallest-first.
    fp8_order = sorted(FP8_CH, key=lambda o: npairs[o])
    order = fp8_order + sorted(bf_ch, key=lambda o: len(blocks[o]))
    # fp8 weights ship as two grouped DMAs (first two channels, then the
    # rest): one DMA per channel loses ~850ns apiece to issue+receipt
    # latency at the stream head, one DMA for all four lands too late.
    w8_groups = [fp8_order[:2], fp8_order[2:]]
    w8_base = {}
    for grp in w8_groups:
        base = 0
        for o in grp:
            w8_base[o] = base
            base += npairs[o]

    nc = bacc.Bacc("TRN2", target_bir_lowering=False, debug=False)
    x8_d = nc.dram_tensor("x8", [128, BPC * xcols], f8, kind="ExternalInput")
    xb_d = nc.dram_tensor("xb", [128, BPC * xcols], bf16, kind="ExternalInput")
    w8_d = [
        nc.dram_tensor(
            f"w8g{g}", [128, sum(npairs[o] for o in grp) * 256], f8,
            kind="ExternalInput",
        )
        for g, grp in enumerate(w8_groups)
    ]
    wb_d = {
        o: nc.dram_tensor(f"wt{o}", [128, len(blocks[o]) * 128], bf16, kind="ExternalInput")
        for o in bf_ch
    }
    y_d = nc.dram_tensor("y", [BPC, OUT_CH, 128, A], bf16, kind="ExternalOutput")

    def dr_weight_ap(wt, o, i):
        a = wt[:]
        off = (w8_base[o] + i) * 256
        return AP(a.tensor, a.offset + off, [list(a.ap[0]), [128, 2], [1, 128]])

    def dr_moving_ap(xt, b, c):
        a = xt[:]
        off = b * xcols + c
        return AP(a.tensor, a.offset + off, [list(a.ap[0]), [1, 2], [1, A]])

    with TileContext(nc) as tc:
        with (
            tc.tile_pool(name="w", bufs=1) as wpool,
            tc.tile_pool(name="x", bufs=1) as xpool,
            tc.tile_pool(name="warm", bufs=1) as warmpool,
            tc.tile_pool(name="ps", bufs=4, space="PSUM") as pspool,
            tc.tile_pool(name="o", bufs=4) as opool,
        ):
            # PE warm-up: dependency-free matmuls bridge the input-DMA wait
            # so HAM un-throttles (1.2 -> 2.4 GHz) before the real stream.
            # The warm-up PSUM tile comes from the ps0 rotation so all 8
            # PSUM banks serve the real chains (4 in flight per batch).
            warm = warmpool.tile([128, 384], bf16, name="warm")
            nc.vector.memset(warm[:], 0.0)
            warm_ps = pspool.tile([128, A], f32, tag="ps0", name="warmps")
            for _ in range(NWARM):
                nc.tensor.matmul(
                    warm_ps[:, :256], warm[:, :128], warm[:, 128:384],
                    start=True, stop=True,
                )
            # DMAs split across both HWDGE rings (sync: x + b0 outputs,
            # scalar: weights + b1 outputs) -- each issue costs ~600ns
            # serially per ring, so the x and first-weight loads must not
            # queue behind each other.
            x8t = xpool.tile([128, BPC * xcols], f8, name="x8")
            xbt = xpool.tile([128, BPC * xcols], bf16, name="xb")
            nc.sync.dma_start(x8t[:], x8_d[:])
            nc.sync.dma_start(xbt[:], xb_d[:])
            w8tiles = []
            for g, grp in enumerate(w8_groups):
                wt = wpool.tile(
                    [128, sum(npairs[o] for o in grp) * 256], f8,
                    tag=f"w8g{g}", name=f"w8g{g}",
                )
                nc.scalar.dma_start(wt[:], w8_d[g][:])
                w8tiles.append(wt)
            w8tile = {o: w8tiles[g] for g, grp in enumerate(w8_groups) for o in grp}
            wtiles = {}
            for o in order:
                if o in FP8_CH:
                    continue
                wt = wpool.tile(
                    [128, len(blocks[o]) * 128], bf16, tag=f"w{o}", name=f"w{o}"
                )
                nc.scalar.dma_start(wt[:], wb_d[o][:])
                wtiles[o] = wt

            for k, o in enumerate(order):
                cs = blocks[o]
                last = k == len(order) - 1
                pss = [
                    pspool.tile([128, A], f32, tag=f"ps{b}", name=f"ps{o}_{b}")
                    for b in range(BPC)
                ]
                # block-major, batches inner: each weight block (or DR
                # pair) loads into the PE array once and streams both
                # batches' columns, keeping LDWEIGHTS fully hidden.
                if o in FP8_CH:
                    for i in range(npairs[o]):
                        for b in range(BPC):
                            nc.tensor.matmul(
                                pss[b][:],
                                dr_weight_ap(w8tile[o], o, i),
                                dr_moving_ap(x8t, b, cs[2 * i]),
                                start=(i == 0),
                                stop=(i == npairs[o] - 1),
                                perf_mode=DR,
                            )
                else:
                    for i, c in enumerate(cs):
                        for b in range(BPC):
                            nc.tensor.matmul(
                                pss[b][:],
                                wtiles[o][:, i * 128 : (i + 1) * 128],
                                xbt[:, b * xcols + c : b * xcols + c + A],
                                start=(i == 0),
                                stop=(i == len(cs) - 1),
                            )
                # b0 casts on vector, b1 casts on scalar (ACT) -- the two
                # cast paths run in parallel.  Output DMAs all go on the
                # sync ring EXCEPT the very last tile's, which rides the
                # then-idle scalar ring so the two final DMAs issue
                # concurrently.  (Putting all b1 DMAs on the scalar ring
                # delays the bf16 weight transfers behind them.)
                scale = float(descale[o]) if o in FP8_CH else 1.0
                for b in range(BPC):
                    ot = opool.tile([128, A], bf16, tag=f"ot{b}", name=f"ot{o}_{b}")
                    if b == 0:
                        if o in FP8_CH:
                            nc.vector.tensor_scalar_mul(ot[:], pss[b][:], scale)
                        else:
                            nc.vector.tensor_copy(ot[:], pss[b][:])
                    else:
                        nc.scalar.activation(
                            ot[:],
                            pss[b][:],
                            mybir.ActivationFunctionType.Copy,
                            scale=scale,
                        )
                    ring = nc.scalar if (last and b == BPC - 1) else nc.sync
                    ring.dma_start(y_d[b, o], ot[:])
    nc.compile()
    return nc


def kernel(inputs, log_omegas):
    global LAST_RESULTS
    import ml_dtypes
    from concourse.bass_utils import run_bass_kernel_spmd

    bf16 = np.dtype(ml_dtypes.bfloat16)
    e4m3 = np.dtype(ml_dtypes.float8_e4m3fn)

    x = np.asarray(inputs, dtype=np.float32).reshape(B, T)
    wbank = _build_wbank(log_omegas)
    blocks = _plan_blocks(wbank)
    cmax = max(c for cs in blocks for c in cs)
    xcols = A + cmax + 1
    assert PADX + T <= xcols * 128, "input padding does not fit block reach"
    wmats, w8mats, descale = _build_weights(wbank, blocks)

    # X[b][u, m] = xpad[b][128*m + u], xpad = [PADX zeros | x | tail zeros]
    xpad = np.zeros((B, xcols * 128), np.float32)
    xpad[:, PADX : PADX + T] = x
    xt_all = np.ascontiguousarray(
        xpad.reshape(B, xcols, 128).transpose(0, 2, 1)
    )  # [B, 128, xcols] f32
    # per core: both batches side by side in the free dim -> [128, BPC*xcols]
    xt_core = np.ascontiguousarray(
        xt_all.reshape(NCORES, BPC, 128, xcols).transpose(0, 2, 1, 3).reshape(
            NCORES, 128, BPC * xcols
        )
    )
    xb_core = xt_core.astype(bf16)
    x8_core = np.clip(xt_core * np.float32(SX), -240, 240).astype(e4m3)

    key = (tuple(tuple(cs) for cs in blocks), xcols, tuple(sorted(descale.items())))
    if key not in _NC_CACHE:
        _NC_CACHE[key] = _build_nc(blocks, xcols, descale)
    nc = _NC_CACHE[key]

    fp8_order = sorted(FP8_CH, key=lambda o: (len(blocks[o]) + 1) // 2)
    w8g = [
        np.ascontiguousarray(np.concatenate([w8mats[o] for o in grp], axis=1))
        for grp in (fp8_order[:2], fp8_order[2:])
    ]
    in_maps = []
    for i in range(NCORES):
        m = {"x8": x8_core[i], "xb": xb_core[i], "w8g0": w8g[0], "w8g1": w8g[1]}
        for o in range(OUT_CH):
            if o not in FP8_CH:
                m[f"wt{o}"] = wmats[o]
        in_maps.append(m)

    res = run_bass_kernel_spmd(
        nc, in_maps, list(range(NCORES)), trace=TRACE, **TRACE_KWARGS
    )
    LAST_RESULTS = res

    # y_dev[b_loc, o, p, a] = y[b, o, 128*a + p]
    y = np.empty((B, OUT_CH, T), np.float32)
    for i in range(NCORES):
        arr = np.asarray(res.results[i]["y"]).astype(np.float32)
        for b in range(BPC):
            y[i * BPC + b] = arr[b].transpose(0, 2, 1).reshape(OUT_CH, T)
    return y.reshape(B, OUT_CH, T)
